# revision 15
# baseline (speedup 1.0000x reference)
"""Distributed Bass kernel for nn_Attention (LN -> QKV -> 16-head attn -> out proj).

Sharding: sequence-split data parallelism over 8 cores, zero collectives.
Core c handles batch c//2 and query-half c%2 (1024 of the 2048 tokens).
K/V are computed redundantly by both cores of a batch pair; attention is
permutation-invariant over keys, so each core receives its batch's tokens
rolled so that its own query half sits at rows [0:1024).

LayerNorm is folded into the QKV matmul:
  qkv = xhat @ w'  +  (-mu*rstd)-row x c-row  +  ones-row x b-row
where xhat = x * rstd (per-token), w' = diag(gamma) @ w_qkv,
c = colsum(w'), b = beta @ w_qkv.

Attention is computed transposed (dots^T [k, q]) so softmax needs no
partition-axis reductions: exp on ScalarE with the 1/8 scale folded in; row
sums come free from a ones-column appended to V (V_aug [k, 65]); the 1/sum
normalization multiplies the PV PSUM during its copy to SBUF, using a DMA
partition-broadcast of the reciprocal sums.

dtypes: QKV projection in bf16 (w', xhat), Q^T/K^T stored f32 and contracted
as float32r (full PE speed at N=512), P@V in bf16, out-proj f32r.
"""

import sys

import numpy as np

sys.path.insert(0, "/opt/trn_rl_repo")

import ml_dtypes
import concourse.bass as bass
import concourse.tile as tile
from concourse import bacc, mybir
from concourse.bass_utils import run_bass_kernel_spmd
from concourse.masks import make_identity

F32 = mybir.dt.float32
F32R = mybir.dt.float32r
BF16 = mybir.dt.bfloat16

T = 2048          # tokens per batch (keys)
TQ = 1024         # own query tokens per core
D = 1024
H = 16
DH = 64
NT = T // 128     # 16 token tiles
NTQ = TQ // 128   # 8 own-query tiles
KD = D // 128     # 8 contraction tiles over d
SCALE = DH ** -0.5

LAST_RESULTS = None


def build_nc():
    nc = bacc.Bacc(trn_type="TRN2")

    x_d = nc.dram_tensor("x", [T, D], F32, kind="ExternalInput")
    wqkv_d = nc.dram_tensor("wqkv", [D, 3 * D], BF16, kind="ExternalInput")
    corr_d = nc.dram_tensor("corr", [2, 3 * D], BF16, kind="ExternalInput")
    wout_d = nc.dram_tensor("wout", [D, D], F32, kind="ExternalInput")
    bout_d = nc.dram_tensor("bout", [1, D], F32, kind="ExternalInput")
    out_d = nc.dram_tensor("out", [TQ, D], F32, kind="ExternalOutput")
    # V_aug staged through DRAM: [ktile, part, head, 65] bf16
    vaug_d = nc.dram_tensor("vaug_scratch", [NT, 128, H, DH + 1], BF16)
    # reciprocal softmax sums bounce through DRAM for partition-broadcast
    srec_d = nc.dram_tensor("srec_scratch", [H, TQ], F32)

    with tile.TileContext(nc) as tc:
        with tc.tile_pool(name="persist", bufs=1) as persist:
            ident = persist.tile([128, 128], BF16)
            make_identity(nc, ident[:])

            # Persistent SBUF tensors
            xhatT = persist.tile([128, KD, T], BF16, tag="bigslot")
            qT = persist.tile([128, KD, TQ], F32)
            kT = persist.tile([128, KD, T], F32, tag="ktslot")
            augT = persist.tile([2, T], BF16)        # row0=-mu*rstd^T, row1=ones
            corr_s = persist.tile([2, 3 * D], BF16)
            ones_f = persist.tile([1, 128], F32)

            # row 1 must be all-ones; row 0 is overwritten by mrT in Phase 1
            nc.vector.memset(augT[:], 1.0)
            ones_tmp = persist.tile([1, 128], F32)
            nc.vector.memset(ones_tmp[:], 1.0)
            nc.vector.tensor_copy(out=ones_f[:].bitcast(F32R), in_=ones_tmp[:])
            nc.sync.dma_start(corr_s[:], corr_d[:])

            # ---------------- Phase 1: LN stats + xhat + transpose ----------
            with tc.tile_pool(name="ln", bufs=2) as ln_pool, \
                 tc.tile_pool(name="lnw", bufs=3) as lnw_pool, \
                 tc.tile_pool(name="tp_ps", bufs=4, space="PSUM") as tp_ps, \
                 tc.tile_pool(name="tm_ps", bufs=2, space="PSUM") as tm_ps:
                eps_t = lnw_pool.tile([128, 1], F32, tag="eps")
                nc.vector.memset(eps_t[:], 1e-5)
                for tt in range(NT):
                    xt = ln_pool.tile([128, D], F32, tag="xt")
                    nc.sync.dma_start(xt[:], x_d[tt * 128:(tt + 1) * 128, :])

                    stats = lnw_pool.tile([128, 2, 6], F32, tag="stats")
                    xg = xt[:].rearrange("p (s f) -> p s f", s=2)
                    for s in range(2):
                        nc.vector.bn_stats(out=stats[:, s, :], in_=xg[:, s, :])
                    mv = lnw_pool.tile([128, 2], F32, tag="mv")
                    nc.vector.bn_aggr(out=mv[:], in_=stats[:])

                    rstd = lnw_pool.tile([128, 1], F32, tag="rstd")
                    nc.scalar.activation(out=rstd[:], in_=mv[:, 1:2],
                                         func=mybir.ActivationFunctionType.Sqrt,
                                         bias=eps_t[:])
                    nc.vector.reciprocal(out=rstd[:], in_=rstd[:])
                    mr = lnw_pool.tile([128, 1], BF16, tag="mr")
                    nc.vector.tensor_scalar(out=mr[:], in0=mv[:, 0:1],
                                            scalar1=rstd[:], scalar2=-1.0,
                                            op0=mybir.AluOpType.mult,
                                            op1=mybir.AluOpType.mult)

                    xhat = ln_pool.tile([128, D], BF16, tag="xhat")
                    nc.vector.tensor_scalar(out=xhat[:], in0=xt[:],
                                            scalar1=rstd[:], scalar2=None,
                                            op0=mybir.AluOpType.mult)

                    for k in range(KD):
                        ps = tp_ps.tile([128, 128], BF16, tag="tps")
                        nc.tensor.transpose(ps[:], xhat[:, k * 128:(k + 1) * 128],
                                            ident[:])
                        nc.vector.tensor_copy(
                            out=xhatT[:, k, tt * 128:(tt + 1) * 128], in_=ps[:])
                    psm = tm_ps.tile([1, 128], BF16, tag="tpm")
                    nc.tensor.transpose(psm[:], mr[:], ident[:])
                    nc.vector.tensor_copy(out=augT[0:1, tt * 128:(tt + 1) * 128],
                                          in_=psm[:])

            # ---------------- Phase 2a: V projection (token-major) ----------
            with tc.tile_pool(name="vw", bufs=10) as vw_pool, \
                 tc.tile_pool(name="vstage", bufs=3) as vstage, \
                 tc.tile_pool(name="v_ps", bufs=3, space="PSUM") as v_ps:
                for nsl in range(2):
                    vw = []
                    for k in range(KD):
                        w = vw_pool.tile([128, 512], BF16, tag="vw")
                        nc.sync.dma_start(
                            out=w[:],
                            in_=wqkv_d[k * 128:(k + 1) * 128,
                                       2 * D + nsl * 512:2 * D + (nsl + 1) * 512])
                        vw.append(w)
                    for tt in range(NT):
                        ps = v_ps.tile([128, 512], F32, tag="vps")
                        for k in range(KD):
                            nc.tensor.matmul(
                                ps[:],
                                lhsT=xhatT[:, k, tt * 128:(tt + 1) * 128],
                                rhs=vw[k][:],
                                start=(k == 0), stop=False)
                        nc.tensor.matmul(
                            ps[:],
                            lhsT=augT[:, tt * 128:(tt + 1) * 128],
                            rhs=corr_s[:, 2 * D + nsl * 512:2 * D + (nsl + 1) * 512],
                            start=False, stop=True)
                        stage = vstage.tile([128, 8, DH + 1], BF16, tag="vst")
                        nc.gpsimd.memset(stage[:, :, DH:DH + 1], 1.0)
                        nc.vector.tensor_copy(
                            out=stage[:, :, 0:DH],
                            in_=ps[:].rearrange("p (h f) -> p h f", h=8))
                        nc.sync.dma_start(
                            out=vaug_d[tt, :, nsl * 8:(nsl + 1) * 8, :],
                            in_=stage[:])

            # ---------------- Phase 2b: Q^T and K^T projections -------------
            with tc.tile_pool(name="qkw", bufs=17) as qkw_pool, \
                 tc.tile_pool(name="qk_ps", bufs=4, space="PSUM") as qk_ps:
                for j in range(16):  # j<8: Q cols, j>=8: K cols
                    wj = []
                    for k in range(KD):
                        w = qkw_pool.tile([128, 128], BF16, tag="qkw")
                        nc.sync.dma_start(
                            out=w[:],
                            in_=wqkv_d[k * 128:(k + 1) * 128,
                                       j * 128:(j + 1) * 128])
                        wj.append(w)
                    is_q = j < 8
                    for ts in range(2 if is_q else 4):
                        ps = qk_ps.tile([128, 512], F32, tag="qkps")
                        for k in range(KD):
                            nc.tensor.matmul(
                                ps[:],
                                lhsT=wj[k][:],
                                rhs=xhatT[:, k, ts * 512:(ts + 1) * 512],
                                start=(k == 0), stop=False)
                        nc.tensor.matmul(
                            ps[:],
                            lhsT=corr_s[:, j * 128:(j + 1) * 128],
                            rhs=augT[:, ts * 512:(ts + 1) * 512],
                            start=False, stop=True)
                        dst = qT if is_q else kT
                        nc.vector.tensor_copy(
                            out=dst[:, j % 8, ts * 512:(ts + 1) * 512].bitcast(F32R),
                            in_=ps[:])

            # ---------------- Phase 3: attention (transposed) ---------------
            outhT = persist.tile([128, KD, TQ], F32, tag="bigslot")
            with tc.tile_pool(name="phat", bufs=18) as phat_pool, \
                 tc.tile_pool(name="vload", bufs=6) as vload, \
                 tc.tile_pool(name="fbc", bufs=4) as fbc, \
                 tc.tile_pool(name="srow", bufs=4) as srow_pool, \
                 tc.tile_pool(name="dots_ps", bufs=2, space="PSUM") as dots_ps, \
                 tc.tile_pool(name="pv_ps", bufs=2, space="PSUM") as pv_ps:
                for p in range(8):  # head pairs (2p, 2p+1)
                    for qsl in range(2):  # own-q slices of 512
                        q0, q1 = qsl * 512, (qsl + 1) * 512
                        pvA = pv_ps.tile([DH + 1, 512], F32, tag="pvA")
                        pvB = pv_ps.tile([DH + 1, 512], F32, tag="pvB")
                        for kt in range(NT):
                            k0, k1 = kt * 128, (kt + 1) * 128
                            dA = dots_ps.tile([128, 512], F32, tag="dA")
                            dB = dots_ps.tile([128, 512], F32, tag="dB")
                            nc.tensor.matmul(
                                dA[:],
                                lhsT=kT[0:DH, p, k0:k1].bitcast(F32R),
                                rhs=qT[0:DH, p, q0:q1].bitcast(F32R),
                                start=True, stop=True)
                            nc.tensor.matmul(
                                dB[:],
                                lhsT=kT[DH:128, p, k0:k1].bitcast(F32R),
                                rhs=qT[DH:128, p, q0:q1].bitcast(F32R),
                                start=True, stop=True,
                                tile_position=(64, 0))
                            pA = phat_pool.tile([128, 512], BF16, tag="phat")
                            pB = phat_pool.tile([128, 512], BF16, tag="phat")
                            nc.scalar.activation(
                                out=pA[:], in_=dA[:],
                                func=mybir.ActivationFunctionType.Exp,
                                scale=SCALE)
                            nc.scalar.activation(
                                out=pB[:], in_=dB[:],
                                func=mybir.ActivationFunctionType.Exp,
                                scale=SCALE)
                            vA = vload.tile([128, DH + 1], BF16, tag="vload")
                            vB = vload.tile([128, DH + 1], BF16, tag="vload")
                            nc.sync.dma_start(out=vA[:], in_=vaug_d[kt, :, 2 * p, :])
                            nc.sync.dma_start(out=vB[:],
                                              in_=vaug_d[kt, :, 2 * p + 1, :])
                            nc.tensor.matmul(pvA[:], lhsT=vA[:], rhs=pA[:],
                                             start=(kt == 0), stop=(kt == NT - 1))
                            nc.tensor.matmul(pvB[:], lhsT=vB[:], rhs=pB[:],
                                             start=(kt == 0), stop=(kt == NT - 1))
                        # 1/rowsum, partition-broadcast, normalize into outhT
                        srA = srow_pool.tile([1, 512], F32, tag="srow")
                        srB = srow_pool.tile([1, 512], F32, tag="srow")
                        nc.vector.reciprocal(out=srA[:], in_=pvA[DH:DH + 1, :])
                        nc.vector.reciprocal(out=srB[:], in_=pvB[DH:DH + 1, :])
                        nc.sync.dma_start(out=srec_d[2 * p:2 * p + 1, q0:q1],
                                          in_=srA[:])
                        nc.sync.dma_start(out=srec_d[2 * p + 1:2 * p + 2, q0:q1],
                                          in_=srB[:])
                        fA = fbc.tile([DH, 512], F32, tag="fbc")
                        fB = fbc.tile([DH, 512], F32, tag="fbc")
                        nc.sync.dma_start(
                            out=fA[:],
                            in_=srec_d[2 * p:2 * p + 1, q0:q1].to_broadcast([DH, 512]))
                        nc.sync.dma_start(
                            out=fB[:],
                            in_=srec_d[2 * p + 1:2 * p + 2, q0:q1].to_broadcast([DH, 512]))
                        nc.vector.tensor_mul(
                            outhT[0:DH, p, q0:q1].bitcast(F32R),
                            pvA[0:DH, :], fA[:])
                        nc.vector.tensor_mul(
                            outhT[DH:128, p, q0:q1].bitcast(F32R),
                            pvB[0:DH, :], fB[:])

            # ---------------- Phase 4: output projection --------------------
            with tc.tile_pool(name="obias", bufs=1) as obias, \
                 tc.tile_pool(name="ostage", bufs=4) as ostage, \
                 tc.tile_pool(name="o_ps", bufs=4, space="PSUM") as o_ps:
                wo = persist.tile([128, KD, D], F32, tag="ktslot")
                for k in range(KD):
                    wst = ostage.tile([128, D], F32, tag="wst")
                    nc.sync.dma_start(out=wst[:],
                                      in_=wout_d[k * 128:(k + 1) * 128, :])
                    nc.vector.tensor_copy(out=wo[:, k, :].bitcast(F32R),
                                          in_=wst[:])
                bo = obias.tile([1, D], F32)
                bst = obias.tile([1, D], F32, tag="bst")
                nc.sync.dma_start(out=bst[:], in_=bout_d[:])
                nc.vector.tensor_copy(out=bo[:].bitcast(F32R), in_=bst[:])

                for qt in range(NTQ):
                    for nsl in range(2):
                        ps = o_ps.tile([128, 512], F32, tag="ops")
                        for k in range(KD):
                            nc.tensor.matmul(
                                ps[:],
                                lhsT=outhT[:, k, qt * 128:(qt + 1) * 128].bitcast(F32R),
                                rhs=wo[:, k, nsl * 512:(nsl + 1) * 512].bitcast(F32R),
                                start=(k == 0), stop=False)
                        nc.tensor.matmul(
                            ps[:],
                            lhsT=ones_f[:].bitcast(F32R),
                            rhs=bo[:, nsl * 512:(nsl + 1) * 512].bitcast(F32R),
                            start=False, stop=True)
                        st = ostage.tile([128, 512], F32, tag="ost")
                        nc.scalar.copy(out=st[:], in_=ps[:])
                        nc.sync.dma_start(
                            out=out_d[qt * 128:(qt + 1) * 128,
                                      nsl * 512:(nsl + 1) * 512],
                            in_=st[:])
    nc.compile()
    return nc


def kernel(x, ln_gamma, ln_beta, w_qkv, w_out, b_out):
    global LAST_RESULTS
    x = np.asarray(x, np.float32)
    ln_gamma = np.asarray(ln_gamma, np.float32)
    ln_beta = np.asarray(ln_beta, np.float32)
    w_qkv = np.asarray(w_qkv, np.float32)
    w_out = np.asarray(w_out, np.float32)
    b_out = np.asarray(b_out, np.float32)

    wq = ln_gamma[:, None] * w_qkv
    corr = np.stack([wq.sum(axis=0), ln_beta @ w_qkv])
    bout2d = np.ascontiguousarray(b_out[None, :])

    nc = build_nc()

    in_maps = []
    for c in range(8):
        b, half = c // 2, c % 2
        xb = np.ascontiguousarray(np.roll(x[b], -half * TQ, axis=0))
        in_maps.append({
            "x": xb,
            "wqkv": np.ascontiguousarray(wq.astype(ml_dtypes.bfloat16)),
            "corr": np.ascontiguousarray(corr.astype(ml_dtypes.bfloat16)),
            "wout": w_out,
            "bout": bout2d,
        })

    res = run_bass_kernel_spmd(nc, in_maps, core_ids=list(range(8)))
    LAST_RESULTS = res

    full = np.empty((4, 2048, D), np.float32)
    for c in range(8):
        b, half = c // 2, c % 2
        full[b, half * TQ:(half + 1) * TQ] = res.results[c]["out"]
    return full
